# revision 8
# baseline (speedup 1.0000x reference)
"""CentroidAttention Trainium2 kernel (8 NeuronCores, SPMD data-parallel over batch).

Reference computation (per problem):
    centers = segment_mean(features, labels, C=1000)       # [C, F]
    q = features @ Wq; k = centers @ Wk; v = centers @ Wv  # [B,A],[C,A],[C,A]
    P = softmax(q @ k.T / sqrt(A))                         # [B, C]
    attn = P @ v @ Wproj + bproj                           # [B, F]
    out = concat([features, attn], -1)                     # [B, 2F]

Sharding: batch B=16384 split 8 ways (2048 rows/core). Each core computes
partial segment sums+counts (as a one-hot matmul, transposed layout
sums.T [F, C]), AllReduce's them, then runs the attention pipeline on its
own batch shard. Weights are replicated.

Device layout choices (all matmuls are out = lhsT.T @ rhs, K on partitions):
  - sums.T [F, C]   <- lhsT = feat chunk [B,F-chunk], rhs = onehot [B, C]
  - feat.T [F, B]   <- PE transposes fused in the segsum pass (same lhsT)
  - q.T   [A, B]    <- lhsT = Wq [F, A], rhs = feat.T
  - kU.T  [A, C]    <- lhsT = Wk [F, A], rhs = sums.T      (unscaled by counts)
  - vU    [C, A]    <- lhsT = sums.T,    rhs = Wv          (unscaled)
  - S.T   [C, B]    <- lhsT = kU.T,      rhs = q.T
  - exp: ACT Exp with per-partition scale = attn_scale * recip_counts[c]
    (folds the centers division of the k-path into the softmax logits)
  - v = vU * recip_counts[c] applied on PSUM evict (folds the v-path division)
  - denom [1, B]    <- lhsT = ones[128,1], rhs = expS.T
  - attnU.T [A, B]  <- lhsT = v [C, A], rhs = expS.T   (unnormalized)
  - outU [B, F]     <- lhsT = attnU.T, rhs = Wproj, plus K=1 row
                       (lhsT=denom-row, rhs=bproj) so bias lands pre-normalized
  - out = outU * recip_denom[b]  (per-partition ACT scale on final evict)

Classes padded 1000 -> 1024 (zero one-hot columns); the padded expS.T rows are
memset to 0 so they contribute nothing to denom or PV.
"""

import numpy as np

import concourse.bass as bass
import concourse.bacc as bacc
import concourse.mybir as mybir
import concourse.tile as tile
from concourse.bass_utils import run_bass_kernel_spmd
from concourse.masks import make_identity

P = 128
B_LOCAL = 2048          # batch rows per core
F = 1024                # feature dim
A = 512                 # attention dim
C = 1000                # num classes
CP = 1024               # classes padded to a multiple of 512
NB = B_LOCAL // P       # 16 batch chunks
NF = F // P             # 8 feature chunks
NA = A // P             # 4 attn-dim chunks
NCC = CP // P           # 8 class chunks
NN = B_LOCAL // 512     # 4 moving-operand chunks over local batch
N_CORES = 8
SCALE = float(A) ** -0.5

F32 = mybir.dt.float32
BF16 = mybir.dt.bfloat16


def _emit(tc):
    nc = tc.nc
    feat_dram = nc.dram_tensor("features", [B_LOCAL, F], F32, kind="ExternalInput")[:]
    lab_dram = nc.dram_tensor("labels_f32", [P, NB], F32, kind="ExternalInput")[:]
    wq_dram = nc.dram_tensor("Wq", [F, A], F32, kind="ExternalInput")[:]
    wk_dram = nc.dram_tensor("Wk", [F, A], F32, kind="ExternalInput")[:]
    wv_dram = nc.dram_tensor("Wv", [F, A], F32, kind="ExternalInput")[:]
    wp_dram = nc.dram_tensor("Wproj", [A, F], F32, kind="ExternalInput")[:]
    bp_dram = nc.dram_tensor("bproj", [1, F], F32, kind="ExternalInput")[:]
    out_dram = nc.dram_tensor("out", [B_LOCAL, F], F32, kind="ExternalOutput")[:]

    from contextlib import ExitStack

    with ExitStack() as ctx:
        consts = ctx.enter_context(tc.tile_pool(name="consts", bufs=1))
        stage = ctx.enter_context(tc.tile_pool(name="stage", bufs=1))
        featn_pool = ctx.enter_context(tc.tile_pool(name="featn", bufs=1))
        p1024 = ctx.enter_context(tc.tile_pool(name="p1024", bufs=1))
        t2048 = ctx.enter_context(tc.tile_pool(name="t2048", bufs=1))
        wpool = ctx.enter_context(tc.tile_pool(name="wpool", bufs=1))
        vpool = ctx.enter_context(tc.tile_pool(name="vpool", bufs=1))
        dram = ctx.enter_context(tc.tile_pool(name="dram", bufs=1, space="DRAM"))

        STAGE_BUFS = 4
        C1024_BUFS = 16
        T2048_BUFS = 12

        def stage_tile(name):
            return stage.tile([P, 1024], F32, name=name, tag="stage", bufs=STAGE_BUFS)

        def c1024_tile(name):
            return p1024.tile([P, CP], BF16, name=name, tag="c1024", bufs=C1024_BUFS)

        def t2048_tile(name):
            return t2048.tile([P, B_LOCAL], BF16, name=name, tag="t2048",
                              bufs=T2048_BUFS)

        # ---- constants ----
        identity = consts.tile([P, P], BF16, name="identity")
        make_identity(nc, identity)
        one1 = consts.tile([1, 1], F32, name="one1")
        nc.gpsimd.memset(one1, 1.0)
        ones_col = consts.tile([P, 1], BF16, name="ones_col")
        nc.gpsimd.memset(ones_col, 1.0)
        iota_g = consts.tile([P, CP], F32, name="iota_g")
        nc.gpsimd.iota(iota_g, pattern=[[1, CP]], base=0, channel_multiplier=0,
                       allow_small_or_imprecise_dtypes=True)
        # funnel iota + labels through DVE: the one-hot tensor_scalar
        # (pointer-scalar variant) only has a single sync-wait slot
        iota = consts.tile([P, CP], F32, name="iota")
        nc.vector.tensor_copy(iota, iota_g)
        labels_ld = consts.tile([P, NB], F32, name="labels_ld")
        nc.sync.dma_start(labels_ld, lab_dram)
        labels_sb = consts.tile([P, NB], F32, name="labels_sb")
        nc.vector.tensor_copy(labels_sb, labels_ld)

        # ---- collective bounce buffers ----
        bounce_in = dram.tile([F + 1, CP], F32, name="bounce_in")
        bounce_out = dram.tile([F + 1, CP], F32, name="bounce_out",
                               addr_space="Shared")

        # ---- phase 0: load features (cast bf16) and build one-hot ----
        feats = []
        for k in range(NB):
            st = stage_tile(f"fst{k}")
            nc.sync.dma_start(st, feat_dram[k * P:(k + 1) * P, :])
            fb = featn_pool.tile([P, F], BF16, name=f"featN{k}")
            nc.vector.tensor_copy(fb, st)
            feats.append(fb)
        onehots = []
        for k in range(NB):
            oh = c1024_tile(f"onehot{k}")
            nc.vector.tensor_scalar(oh, iota, labels_sb[:, k:k + 1], None,
                                    mybir.AluOpType.is_equal)
            onehots.append(oh)

        # ---- phase A: counts = ones.T @ onehot  -> bounce row F ----
        with tc.tile_pool(name="pcnt", bufs=1, space="PSUM") as pcnt:
            cps = pcnt.tile([1, CP], F32, name="counts_ps")
            for k in range(NB):
                for h in range(2):
                    nc.tensor.matmul(cps[:, h * 512:(h + 1) * 512],
                                     lhsT=ones_col,
                                     rhs=onehots[k][:, h * 512:(h + 1) * 512],
                                     start=(k == 0), stop=(k == NB - 1))
            cnt_sb = consts.tile([1, CP], F32, name="cnt_sb")
            nc.vector.tensor_copy(cnt_sb, cps)
            nc.sync.dma_start(bounce_in[F:F + 1, :], cnt_sb)

        # ---- phase B: segment sums (transposed) + feat.T via fused PE transpose
        featTs = []
        with tc.tile_pool(name="pseg", bufs=1, space="PSUM") as pseg:
            for j in range(NF):
                sps = pseg.tile([P, CP], F32, name=f"sums{j}", tag="sums", bufs=2)
                ftA = pseg.tile([P, F], BF16, name=f"ftA{j}", tag="ftA", bufs=1)
                ftB = pseg.tile([P, F], BF16, name=f"ftB{j}", tag="ftB", bufs=1)
                for k in range(NB):
                    lhsT = feats[k][:, j * P:(j + 1) * P]
                    for h in range(2):
                        nc.tensor.matmul(sps[:, h * 512:(h + 1) * 512],
                                         lhsT=lhsT,
                                         rhs=onehots[k][:, h * 512:(h + 1) * 512],
                                         start=(k == 0), stop=(k == NB - 1))
                    ft = ftA if k < 8 else ftB
                    nc.tensor.transpose(ft[:, (k % 8) * P:(k % 8 + 1) * P], lhsT,
                                        identity)
                ftile = t2048_tile(f"featT{j}")
                nc.vector.tensor_copy(ftile[:, 0:F], ftA)
                nc.vector.tensor_copy(ftile[:, F:2 * F], ftB)
                featTs.append(ftile)
                sums_sb = stage_tile(f"sums_sb{j}")
                nc.scalar.copy(sums_sb, sps)
                nc.sync.dma_start(bounce_in[j * P:(j + 1) * P, :], sums_sb)

        # ---- AllReduce partial sums + counts across the 8 cores ----
        nc.gpsimd.collective_compute(
            "AllReduce", mybir.AluOpType.add,
            replica_groups=[list(range(N_CORES))],
            ins=[bounce_in.opt()], outs=[bounce_out.opt()],
        )

        # ---- weights: load + cast (overlaps the collective) ----
        wqb, wkb, wvb = [], [], []
        for nm, src, dst in (("wq", wq_dram, wqb), ("wk", wk_dram, wkb),
                             ("wv", wv_dram, wvb)):
            for j in range(NF):
                st = stage_tile(f"{nm}st{j}")
                nc.sync.dma_start(st[:, 0:A], src[j * P:(j + 1) * P, :])
                wb = wpool.tile([P, A], BF16, name=f"{nm}b{j}")
                nc.vector.tensor_copy(wb, st[:, 0:A])
                dst.append(wb)
        wpb = []
        for a in range(NA):
            st = stage_tile(f"wpst{a}")
            nc.sync.dma_start(st, wp_dram[a * P:(a + 1) * P, :])
            wb = wpool.tile([P, F], BF16, name=f"wpb{a}")
            nc.vector.tensor_copy(wb, st)
            wpb.append(wb)
        bst = stage_tile("bst")
        nc.sync.dma_start(bst[0:1, :], bp_dram)
        bprojb = wpool.tile([1, F], BF16, name="bprojb")
        nc.vector.tensor_copy(bprojb, bst[0:1, :])

        # ---- q.T = Wq.T @ feat.T (PE busy during the collective) ----
        qTs = []
        with tc.tile_pool(name="pq", bufs=1, space="PSUM") as pq:
            for a in range(NA):
                qps = pq.tile([P, B_LOCAL], F32, name=f"qps{a}", tag="q", bufs=2)
                for n in range(NN):
                    for j in range(NF):
                        nc.tensor.matmul(qps[:, n * 512:(n + 1) * 512],
                                         lhsT=wqb[j][:, a * P:(a + 1) * P],
                                         rhs=featTs[j][:, n * 512:(n + 1) * 512],
                                         start=(j == 0), stop=(j == NF - 1))
                qt = t2048_tile(f"qT{a}")
                nc.vector.tensor_copy(qt, qps)
                qTs.append(qt)

        # ---- read back reduced sums + counts ----
        sumsb = []
        for j in range(NF):
            st = stage_tile(f"sst{j}")
            nc.sync.dma_start(st, bounce_out[j * P:(j + 1) * P, :])
            sb = c1024_tile(f"sumsb{j}")
            nc.vector.tensor_copy(sb, st)
            sumsb.append(sb)
        counts_sb = consts.tile([1, CP], F32, name="counts_sb")
        nc.sync.dma_start(counts_sb, bounce_out[F:F + 1, :])

        kTs, vbs = [], []
        with tc.tile_pool(name="pkv", bufs=1, space="PSUM") as pkv:
            # recip_counts in [C-chunk(partition), chunk-idx] layout
            cpsT = pkv.tile([P, NCC], F32, name="cntT")
            for c in range(NCC):
                nc.tensor.transpose(cpsT[:, c:c + 1],
                                    counts_sb[:, c * P:(c + 1) * P], one1)
            cnt_m = consts.tile([P, NCC], F32, name="cnt_m")
            nc.vector.tensor_scalar_max(cnt_m, cpsT, 1.0)
            recip_cols = consts.tile([P, NCC], F32, name="recip_cols")
            nc.vector.reciprocal(recip_cols, cnt_m)
            exp_scale = consts.tile([P, NCC], F32, name="exp_scale")
            nc.vector.tensor_scalar_mul(exp_scale, recip_cols, SCALE)

            # kU.T [A, C] ; counts division folded into the exp scale later
            for a in range(NA):
                kps = pkv.tile([P, CP], F32, name=f"kps{a}", tag="k", bufs=2)
                for h in range(2):
                    for j in range(NF):
                        nc.tensor.matmul(kps[:, h * 512:(h + 1) * 512],
                                         lhsT=wkb[j][:, a * P:(a + 1) * P],
                                         rhs=sumsb[j][:, h * 512:(h + 1) * 512],
                                         start=(j == 0), stop=(j == NF - 1))
                kt = c1024_tile(f"kT{a}")
                nc.vector.tensor_copy(kt, kps)
                kTs.append(kt)

            # v [C, A] = (sums.T).T @ Wv, scaled by recip_counts on evict
            for c in range(NCC):
                vps = pkv.tile([P, A], F32, name=f"vps{c}", tag="v", bufs=2)
                for j in range(NF):
                    nc.tensor.matmul(vps,
                                     lhsT=sumsb[j][:, c * P:(c + 1) * P],
                                     rhs=wvb[j],
                                     start=(j == 0), stop=(j == NF - 1))
                vb = vpool.tile([P, A], BF16, name=f"vb{c}")
                nc.vector.tensor_scalar_mul(vb, vps, recip_cols[:, c:c + 1])
                vbs.append(vb)

        # ---- S.T [C, B] and exp (centers division folded into scale) ----
        expSTs = []
        with tc.tile_pool(name="pst", bufs=1, space="PSUM") as pst:
            for c in range(NCC):
                sps = pst.tile([P, B_LOCAL], F32, name=f"stps{c}", tag="st", bufs=2)
                for n in range(NN):
                    for a in range(NA):
                        nc.tensor.matmul(sps[:, n * 512:(n + 1) * 512],
                                         lhsT=kTs[a][:, c * P:(c + 1) * P],
                                         rhs=qTs[a][:, n * 512:(n + 1) * 512],
                                         start=(a == 0), stop=(a == NA - 1))
                est = t2048_tile(f"expST{c}")
                rows = (C - c * P) if c == NCC - 1 else P
                if rows < P:
                    # zero the padded class rows; exp overwrites the valid ones
                    nc.vector.memset(est, 0.0)
                nc.scalar.activation(est[0:rows, :], sps[0:rows, :],
                                     mybir.ActivationFunctionType.Exp,
                                     bias=0.0, scale=exp_scale[0:rows, c:c + 1])
                expSTs.append(est)

        # ---- softmax denominator [1, B] + its reciprocal transposed ----
        recipD_cols = consts.tile([P, NB], F32, name="recipD_cols")
        den_b = consts.tile([1, B_LOCAL], BF16, name="den_b")
        with tc.tile_pool(name="pden", bufs=1, space="PSUM") as pden:
            dps = pden.tile([1, B_LOCAL], F32, name="dps")
            for n in range(NN):
                for c in range(NCC):
                    nc.tensor.matmul(dps[:, n * 512:(n + 1) * 512],
                                     lhsT=ones_col,
                                     rhs=expSTs[c][:, n * 512:(n + 1) * 512],
                                     start=(c == 0), stop=(c == NCC - 1))
            recipD = consts.tile([1, B_LOCAL], F32, name="recipD")
            nc.vector.reciprocal(recipD, dps)
            nc.vector.tensor_copy(den_b, dps)
            rdps = pden.tile([P, NB], F32, name="rdps")
            for t in range(NB):
                nc.tensor.transpose(rdps[:, t:t + 1],
                                    recipD[:, t * P:(t + 1) * P], one1)
            nc.vector.tensor_copy(recipD_cols, rdps)

        # ---- attnU.T [A, B] = v.T @ expS.T (unnormalized) ----
        attnTs = []
        with tc.tile_pool(name="ppv", bufs=1, space="PSUM") as ppv:
            for a in range(NA):
                aps = ppv.tile([P, B_LOCAL], F32, name=f"aps{a}", tag="av", bufs=2)
                for n in range(NN):
                    for c in range(NCC):
                        nc.tensor.matmul(aps[:, n * 512:(n + 1) * 512],
                                         lhsT=vbs[c][:, a * P:(a + 1) * P],
                                         rhs=expSTs[c][:, n * 512:(n + 1) * 512],
                                         start=(c == 0), stop=(c == NCC - 1))
                at = t2048_tile(f"attnT{a}")
                nc.vector.tensor_copy(at, aps)
                attnTs.append(at)

        # ---- out = (attnU.T.T @ Wproj + denom*bproj) * recip_denom ----
        with tc.tile_pool(name="po", bufs=1, space="PSUM") as po:
            for t in range(NB):
                ops = po.tile([P, F], F32, name=f"ops{t}", tag="o", bufs=2)
                for h in range(2):
                    for a in range(NA):
                        nc.tensor.matmul(ops[:, h * 512:(h + 1) * 512],
                                         lhsT=attnTs[a][:, t * P:(t + 1) * P],
                                         rhs=wpb[a][:, h * 512:(h + 1) * 512],
                                         start=(a == 0), stop=False)
                    nc.tensor.matmul(ops[:, h * 512:(h + 1) * 512],
                                     lhsT=den_b[:, t * P:(t + 1) * P],
                                     rhs=bprojb[:, h * 512:(h + 1) * 512],
                                     start=False, stop=True)
                osb = stage_tile(f"osb{t}")
                nc.scalar.activation(osb, ops,
                                     mybir.ActivationFunctionType.Copy,
                                     bias=0.0, scale=recipD_cols[:, t:t + 1])
                nc.sync.dma_start(out_dram[t * P:(t + 1) * P, :], osb)


_BUILT = None


def _get_nc():
    global _BUILT
    if _BUILT is None:
        nc = bacc.Bacc("TRN2", target_bir_lowering=False, debug=False,
                       num_devices=N_CORES)
        with tile.TileContext(nc) as tc:
            _emit(tc)
        nc.compile()
        _BUILT = nc
    return _BUILT


def _make_in_maps(inputs):
    features = np.ascontiguousarray(np.asarray(inputs["features"],
                                               dtype=np.float32))
    labels = np.ascontiguousarray(np.asarray(inputs["labels"])).astype(np.int64)
    Wq = np.ascontiguousarray(np.asarray(inputs["Wq"], dtype=np.float32))
    Wk = np.ascontiguousarray(np.asarray(inputs["Wk"], dtype=np.float32))
    Wv = np.ascontiguousarray(np.asarray(inputs["Wv"], dtype=np.float32))
    Wproj = np.ascontiguousarray(np.asarray(inputs["Wproj"], dtype=np.float32))
    bproj = np.ascontiguousarray(
        np.asarray(inputs["bproj"], dtype=np.float32)).reshape(1, F)

    in_maps = []
    for cix in range(N_CORES):
        fl = features[cix * B_LOCAL:(cix + 1) * B_LOCAL]
        ll = labels[cix * B_LOCAL:(cix + 1) * B_LOCAL]
        lab2d = np.ascontiguousarray(
            ll.astype(np.float32).reshape(NB, P).T)
        in_maps.append({
            "features": fl,
            "labels_f32": lab2d,
            "Wq": Wq, "Wk": Wk, "Wv": Wv, "Wproj": Wproj, "bproj": bproj,
        })
    return in_maps


def _assemble(inputs, results):
    features = np.asarray(inputs["features"], dtype=np.float32)
    out = np.empty((N_CORES * B_LOCAL, 2 * F), np.float32)
    out[:, :F] = features
    for cix in range(N_CORES):
        out[cix * B_LOCAL:(cix + 1) * B_LOCAL, F:] = results[cix]["out"]
    return out


def _run(inputs, **run_kwargs):
    nc = _get_nc()
    in_maps = _make_in_maps(inputs)
    res = run_bass_kernel_spmd(nc, in_maps, list(range(N_CORES)), **run_kwargs)
    return _assemble(inputs, res.results), res


def kernel(**inputs):
    out, _ = _run(inputs)
    return out
